# revision 5
# baseline (speedup 1.0000x reference)
"""Channel-wise (XCA / XCiT-style) self-attention Trainium2 kernel.

Problem: x:(8,192,128,128) -> qkv proj -> per-head (d=24) channel attention
over N=16384 spatial positions with L2-normalized q,k -> out proj.

Sharding: data-parallel over batch B=8, one batch per NeuronCore (8 cores).
Each core runs an identical single-core program on its x[b] slice (C,N) and
produces out[b] (C,N); the host stacks the results.

Per-core dataflow (C=192 channels, N=16384, 8 heads x d=24, 2 head-groups
of 96 channels):
  pass 1 (fp8e4m3, DoubleRow matmuls = 2 rows/cycle on the PE):
    - [q|k] = x^T Wqk in (N,2C) layout. x is shipped as x8[97,2,N] fp8 with
      channel c = t*96+p; row p=96 is (ones, zeros) which folds the qkv bias
      (stored in the matching wqk8 row) into the projection. One DoubleRow
      matmul per 128-spatial subtile contracts all 194 rows.
      Wqk columns are host-permuted group-major (col = g*192 + {q,k}*96 + i)
      so every gram operand below is a contiguous column range (the
      double-row ISA allows at most 2 free dims on the moving operand).
    - PSUM -> SBUF evacuation is a pure fp32->fp8 cast, alternated between
      the DVE and ACT engines, into qk8 tiles [128, 2, 384] that pair two
      consecutive subtiles in the double-row k-tile dim.
    - S^T[e,d] = k.q gram blocks + diag(q.q), diag(k.k) accumulated in PSUM
      over all N: per subtile-pair one DoubleRow gram (contracting 256
      spatial rows) per {S^T|k-diag, q-diag} x 2 groups. The gram matmuls
      for pair u are issued on the PE after the projections for pair u+2
      (software pipelining) so the PE never stalls on the evacuations and
      stays at its top p-state clock.
  softmax phase (tiny, fp32): rq=1/max(sqrt(diag q.q),eps) (x temperature),
    rk likewise; S^T*rk -PE-transpose-> S*rq_t + block-diag mask -> softmax
    rows -> A; then the output projection is folded into the attention
    weights: Wt_g[e,c'] = sum_d A_g[d,e] Wp_g[d,c']  (one tiny matmul/group)
  pass 2 (fp16 for output precision): out[c',n] = Wfused^T x + bias_tot.
    v is never materialized: Wfused[c,c'] = Wv (A Wp) is built on-device;
    fp16 x stays resident in SBUF from its own DMA (x0[128,N] + x1[65,N]
    whose last row is ones, folding bias_tot cheaply is still done via the
    ACT bias port since it is per-partition here).

Errors stay tiny because fp8 only touches q/k: attention logits are cosines
of 16384-long vectors, so the 3.6% rms fp8 quantization error attenuates by
1/sqrt(N) in the logits; the v/output path stays fp16.

DMA: x8 + x0 on the SP HWDGE queue, x1 on the ACT queue; x is resident in
SBUF as per-chunk tiles so the repeat-loop iteration i+1 can prefetch chunk
0 while iteration i is still in pass 2 on later chunks.
"""

import numpy as np

B, C, HH, WW = 8, 192, 128, 128
N = HH * WW
NHEADS, DH, G, GC = 8, 24, 2, 96
NEG_BIG = -1.0e30

_BUILT = {}


def _patch_tile_drain():
    """The final TileContext drain carries one sem wait per live processor;
    this container's walrus codegen only accepts a single sync wait on the
    CTRL Drain. Split the waits across a chain of drains (1 wait each)."""
    import bass_rust
    import concourse.tile as tile
    from concourse.vector_clock import ScopedClock

    if getattr(tile.TileContext, "_drain_split_patch", False):
        return

    def _split_drain_and_barrier(self, tick_clock, wait_clock):
        nc = self.nc
        drain_bi = nc.sync.drain()
        wait_clock.add_sem_waits(
            drain_bi.ins, ScopedClock({None: tick_clock.global_clock})
        )
        inst = drain_bi.ins
        si = inst.sync_info
        if si is not None:
            waits = list(si.on_wait or [])
            ups = list(si.on_update or [])
            if len(waits) > 1:
                inst.sync_info = bass_rust.SyncInfo(on_wait=[waits[0]], on_update=[])
                for i, w in enumerate(waits[1:]):
                    extra = nc.sync.drain()
                    last = i == len(waits) - 2
                    extra.ins.sync_info = bass_rust.SyncInfo(
                        on_wait=[w], on_update=ups if last else []
                    )
        nc.all_engine_barrier()
        assert self.sems is not None
        popped = nc._tile_sem_poison_stack.pop()
        assert popped is self._sem_poison
        nc.clear_and_free_semaphores(list(self.sems.allocated().values()))
        nc.all_engine_barrier()

    tile.TileContext._drain_and_barrier = _split_drain_and_barrier
    tile.TileContext._drain_split_patch = True


def _split_excess_waits(nc, max_waits=1):
    """This container's walrus codegen accepts at most one sync wait per
    instruction. Move excess waits onto NoOp carriers inserted just before
    the instruction on the same engine (engine streams process waits in
    issue order, so this is semantics-preserving)."""
    import bass_rust
    from concourse import mybir

    for f in nc.m.functions:
        for bb in f.blocks:
            insts = bb.instructions
            if not any(
                getattr(i, "sync_info", None) is not None
                and i.sync_info.on_wait
                and len(list(i.sync_info.on_wait)) > max_waits
                for i in insts
            ):
                continue
            newlist = []
            for inst in insts:
                si = getattr(inst, "sync_info", None)
                if si is not None and si.on_wait:
                    waits = list(si.on_wait)
                    if len(waits) > max_waits:
                        keep = waits[-max_waits:]
                        for wi, w in enumerate(waits[: -max_waits]):
                            es = mybir.InstNoOp(
                                name=f"{inst.name}-xw{wi}", ins=[], outs=[]
                            )
                            es.engine = inst.engine
                            es.sync_info = bass_rust.SyncInfo(
                                on_wait=[w], on_update=[]
                            )
                            newlist.append(es)
                        inst.sync_info = bass_rust.SyncInfo(
                            on_wait=keep, on_update=list(si.on_update or [])
                        )
                newlist.append(inst)
            bb.instructions = newlist


def _build(n_total=N, split=True, repeat=1, stages=("v", "qk", "s", "sm", "p2")):
    """Build the single-core Bass program. Returns nc."""
    import contextlib as _ctxlib

    import concourse.bass as bass
    import concourse.tile as tile
    from concourse import mybir

    _patch_tile_drain()

    f32 = mybir.dt.float32
    f16 = mybir.dt.float16
    f8 = mybir.dt.float8e4
    AFT = mybir.ActivationFunctionType
    ALU = mybir.AluOpType
    AX = mybir.AxisListType
    DR = mybir.MatmulPerfMode.DoubleRow

    BW = 512  # compute block width
    CW = 4096 if n_total % 4096 == 0 else 512  # DMA staging chunk width
    NCH = n_total // CW  # staging chunks
    BPC = CW // BW  # compute blocks per chunk
    SUB = BW // 128  # 128-row n-subtiles per block
    NSUB = n_total // 128  # total 128-row subtiles
    NPAIR = NSUB // 2  # subtile pairs (double-row gram granularity)
    PIPE = 3  # software-pipeline lag (subtiles) between proj and gram

    nc = bass.Bass("TRN2", target_bir_lowering=False, debug=False)

    # fp8 x for pass 1: [97 p, 2 t, N], channel c = t*96+p, row 96 = (1, 0)
    x8d = nc.dram_tensor("x8", [97, 2, n_total], f8, kind="ExternalInput").ap()
    # fp16 x for pass 2
    x0d = nc.dram_tensor("x0", [128, n_total], f16, kind="ExternalInput").ap()
    x1d = nc.dram_tensor("x1", [65, n_total], f16, kind="ExternalInput").ap()
    # Wqk fp8, rows like x8 (row 96 = (bias, 0)), cols group-major permuted
    wqk8 = nc.dram_tensor("wqk8", [97, 2, 2 * C], f8, kind="ExternalInput").ap()
    wvt = nc.dram_tensor("wvt", [GC, G, C], f16, kind="ExternalInput").ap()
    bv16 = nc.dram_tensor("bv16", [GC, G], f16, kind="ExternalInput").ap()
    wp0 = nc.dram_tensor("wp0", [GC, C], f16, kind="ExternalInput").ap()
    wp1 = nc.dram_tensor("wp1", [GC, C], f16, kind="ExternalInput").ap()
    bp = nc.dram_tensor("bp", [GC, G], f32, kind="ExternalInput").ap()
    tmp96 = nc.dram_tensor("tmp96", [GC, G], f32, kind="ExternalInput").ap()
    eye96 = nc.dram_tensor("eye96", [GC, GC], f32, kind="ExternalInput").ap()
    bmask = nc.dram_tensor("bmask", [GC, GC], f32, kind="ExternalInput").ap()
    out = nc.dram_tensor("out", [C, n_total], f16, kind="ExternalOutput").ap()

    with tile.TileContext(nc) as tc:
        with (
            tc.tile_pool(name="const", bufs=1) as const,
            tc.tile_pool(name="qkp", bufs=6) as qkp,
            tc.tile_pool(name="vres", bufs=1) as vres,
            tc.tile_pool(name="small", bufs=1) as small,
            tc.tile_pool(name="op", bufs=2) as op,
            tc.tile_pool(name="psA", bufs=5, space="PSUM") as psA,
            tc.tile_pool(name="psB", bufs=1, space="PSUM") as psB,
            tc.tile_pool(name="psS", bufs=1, space="PSUM") as psS,
        ):
            # --- constants into SBUF (first-use order) ---
            wqk8_sb = const.tile([97, 2, 2 * C], f8)
            nc.sync.dma_start(wqk8_sb[:], wqk8)

            # --- persistent x tiles, one set per chunk (pass 2 re-reads) ---
            x8_sb = [
                vres.tile([97, 2, CW], f8, tag=f"x8r{ch}", name=f"x8r{ch}")
                for ch in range(NCH)
            ]
            x0_sb = [
                vres.tile([128, CW], f16, tag=f"x0r{ch}", name=f"x0r{ch}")
                for ch in range(NCH)
            ]
            x1_sb = [
                vres.tile([65, CW], f16, tag=f"x1r{ch}", name=f"x1r{ch}")
                for ch in range(NCH)
            ]
            # gram accumulators: [e(96), {S^T | k-diag | q-diag}, 96]
            # (one PSUM bank per group)
            sg_ps = [
                psS.tile([GC, 3, GC], f32, tag=f"SG{g}", name=f"SG{g}")
                for g in range(G)
            ]

            _rep_cm = (
                tc.For_i(0, repeat, 1, hint_engines=tuple(nc.engines.keys()))
                if repeat > 1
                else _ctxlib.nullcontext()
            )
            with _rep_cm:
                # ---------------- pass 1 ----------------
                # proj for subtile j lands in qk8 tile j//2 slot j%2; gram for
                # pair u is issued PIPE subtiles behind.
                pend = []  # (qk8_tile, u) pairs awaiting gram emission

                def emit_gram(qk, u):
                    first = u == 0
                    last = u == NPAIR - 1
                    for g in range(G):
                        qg = qk[:, :, g * 192 : g * 192 + 96]
                        kg = qk[:, :, g * 192 + 96 : g * 192 + 192]
                        qkg = qk[:, :, g * 192 : g * 192 + 192]
                        # S^T + k-diag: lhsT = k_g, rhs = [q_g | k_g]
                        nc.tensor.matmul(
                            sg_ps[g][:, 0:2, :],
                            kg,
                            qkg,
                            start=first,
                            stop=last,
                            perf_mode=DR,
                            skip_group_check=True,
                        )
                        # q-diag: lhsT = q_g, rhs = q_g
                        nc.tensor.matmul(
                            sg_ps[g][:, 2, :],
                            qg,
                            qg,
                            start=first,
                            stop=last,
                            perf_mode=DR,
                            skip_group_check=True,
                        )

                qk_cur = None
                for ch in range(NCH):
                    if ch == 0:
                        # fine-grained first chunk so the PE starts ~8x sooner
                        for q in range(BPC):
                            qs = slice(q * BW, (q + 1) * BW)
                            nc.sync.dma_start(
                                x8_sb[ch][:, :, qs], x8d[:, :, qs]
                            )
                        for q in range(BPC):
                            qs = slice(q * BW, (q + 1) * BW)
                            nc.sync.dma_start(x0_sb[ch][:, qs], x0d[:, qs])
                            nc.scalar.dma_start(x1_sb[ch][:, qs], x1d[:, qs])
                    else:
                        cs = slice(ch * CW, (ch + 1) * CW)
                        nc.sync.dma_start(x8_sb[ch][:], x8d[:, :, cs])
                        nc.sync.dma_start(x0_sb[ch][:], x0d[:, cs])
                        nc.scalar.dma_start(x1_sb[ch][:], x1d[:, cs])

                    for bi in range(BPC):
                        for j in range(SUB if "qk" in stages else 0):
                            blk = ch * BPC + bi
                            jg = blk * SUB + j  # global subtile index
                            js = slice(bi * BW + j * 128, bi * BW + (j + 1) * 128)
                            pqk = psA.tile([128, 2 * C], f32, tag="A")
                            nc.tensor.matmul(
                                pqk[:],
                                x8_sb[ch][:, :, js],
                                wqk8_sb[:],
                                start=True,
                                stop=True,
                                perf_mode=DR,
                            )
                            if jg % 2 == 0:
                                qk_cur = qkp.tile([128, 2, 2 * C], f8, tag="qk")
                            # pure fp32->fp8 cast (bias folded into ones-row),
                            # alternated between DVE and ACT
                            if "nocopy" not in stages:
                                if jg % 2 == 0:
                                    nc.vector.tensor_copy(
                                        out=qk_cur[:, jg % 2, :], in_=pqk[:]
                                    )
                                else:
                                    nc.scalar.activation(
                                        out=qk_cur[:, jg % 2, :],
                                        in_=pqk[:],
                                        func=AFT.Identity,
                                    )
                            else:
                                nc.vector.memset(qk_cur[:, jg % 2, 0:1], 0.0)
                            if "s" in stages:
                                if jg % 2 == 1:
                                    pend.append((qk_cur, jg // 2))
                                if len(pend) * 2 > PIPE:
                                    emit_gram(*pend.pop(0))
                for qk, u in pend:
                    emit_gram(qk, u)
                pend = []

                # --- softmax/pass-2 constants (not needed until pass 1 ends) ---
                wp0_sb = const.tile([GC, C], f16)
                nc.sync.dma_start(wp0_sb[:], wp0)
                wp1_sb = const.tile([GC, C], f16)
                nc.sync.dma_start(wp1_sb[:], wp1)
                bp_sb = const.tile([GC, G], f32)
                nc.sync.dma_start(bp_sb[:], bp)
                tmp96_sb = const.tile([GC, G], f32)
                nc.sync.dma_start(tmp96_sb[:], tmp96)
                eye_sb = const.tile([GC, GC], f32)
                nc.sync.dma_start(eye_sb[:], eye96)
                bmask_sb = const.tile([GC, GC], f32)
                nc.sync.dma_start(bmask_sb[:], bmask)

                # ---------------- softmax phase ----------------
                wt_sb = []
                if "sm" not in stages and "p2" in stages:
                    for g in range(G):
                        wtd = small.tile([GC, C], f16, tag=f"wt{g}", name=f"wtd{g}")
                        nc.vector.memset(wtd[:], 0.001)
                        wt_sb.append(wtd)
                for g in range(G if "sm" in stages else 0):
                    trash = small.tile([GC, GC], f32, tag="trash")
                    kss = small.tile([GC, 1], f32, tag=f"kss{g}")
                    nc.vector.tensor_mul(
                        out=trash[:], in0=sg_ps[g][:, 1, :], in1=eye_sb[:]
                    )
                    nc.vector.reduce_sum(out=kss[:], in_=trash[:], axis=AX.X)
                    trash2 = small.tile([GC, GC], f32, tag="trash")
                    qss = small.tile([GC, 1], f32, tag=f"qss{g}")
                    nc.vector.tensor_mul(
                        out=trash2[:], in0=sg_ps[g][:, 2, :], in1=eye_sb[:]
                    )
                    nc.vector.reduce_sum(out=qss[:], in_=trash2[:], axis=AX.X)
                    # r = 1 / max(sqrt(ss), eps)
                    for ss in (kss, qss):
                        nc.scalar.sqrt(ss[:], ss[:])
                        nc.vector.tensor_scalar_max(out=ss[:], in0=ss[:], scalar1=1e-12)
                        nc.vector.reciprocal(ss[:], ss[:])
                    # fold temperature into rq
                    nc.vector.tensor_tensor(
                        out=qss[:], in0=qss[:], in1=tmp96_sb[:, g, None], op=ALU.mult
                    )
                    # S^T scaled by rk (rows = e)
                    st_sb = small.tile([GC, GC], f32, tag="st")
                    nc.vector.tensor_scalar_mul(
                        out=st_sb[:], in0=sg_ps[g][:, 0, :], scalar1=kss[:]
                    )
                    # transpose -> S (rows = d)
                    ps_tr = psA.tile([GC, GC], f32, tag="A")
                    nc.tensor.transpose(ps_tr[:], st_sb[:], eye_sb[:])
                    s_sb = small.tile([GC, GC], f32, tag="s")
                    nc.vector.tensor_scalar_mul(
                        out=s_sb[:], in0=ps_tr[:], scalar1=qss[:]
                    )
                    nc.vector.tensor_tensor(
                        out=s_sb[:], in0=s_sb[:], in1=bmask_sb[:], op=ALU.add
                    )
                    # softmax rows
                    nmax = small.tile([GC, 1], f32, tag=f"nmax{g}")
                    nc.vector.reduce_max(
                        out=nmax[:], in_=s_sb[:], axis=AX.X, negate=True
                    )
                    e_sb = small.tile([GC, GC], f32, tag="e")
                    rsum = small.tile([GC, 1], f32, tag=f"rsum{g}")
                    nc.scalar.activation(
                        out=e_sb[:],
                        in_=s_sb[:],
                        func=AFT.Exp,
                        bias=nmax[:],
                        scale=1.0,
                        accum_out=rsum[:],
                    )
                    nc.vector.reciprocal(rsum[:], rsum[:])
                    a_sb = small.tile([GC, GC], f16, tag="a")
                    nc.vector.tensor_scalar_mul(
                        out=a_sb[:], in0=e_sb[:], scalar1=rsum[:]
                    )
                    # fold output projection: Wt_g[e,c'] = sum_d A_g[d,e] Wp_g[d,c']
                    ps_w = psA.tile([GC, C], f32, tag="A")
                    nc.tensor.matmul(
                        ps_w[:],
                        a_sb[:],
                        (wp0_sb if g == 0 else wp1_sb)[:],
                        start=True,
                        stop=True,
                    )
                    wt = small.tile([GC, C], f16, tag=f"wt{g}")
                    nc.scalar.activation(out=wt[:], in_=ps_w[:], func=AFT.Identity)
                    wt_sb.append(wt)

                # Wfused[c,c'] = sum_g sum_e Wv[c,96g+e] Wt_g[e,c']  and
                # bias_tot[c'] = sum_g Wt_g^T bv_g + bp  -> pass 2 is just
                # out = Wfused^T x + bias_tot.
                if "p2" in stages:
                    wvt_sb = const.tile([GC, G, C], f16, name="wvt_sb")
                    nc.sync.dma_start(wvt_sb[:], wvt)
                    bv16_sb = const.tile([GC, G], f16, name="bv16_sb")
                    nc.sync.dma_start(bv16_sb[:], bv16)
                    wf_sb = []
                    for kc, (p0, sz) in enumerate(((0, 128), (128, 64))):
                        ps_wf = psA.tile([128, C], f32, tag="A", name=f"pswf{kc}")
                        for g in range(G):
                            nc.tensor.matmul(
                                ps_wf[:sz, :],
                                wvt_sb[:, g, p0 : p0 + sz],
                                wt_sb[g][:],
                                start=(g == 0),
                                stop=(g == G - 1),
                            )
                        wf = small.tile([128, C], f16, tag=f"wf{kc}", name=f"wf{kc}")
                        nc.scalar.activation(
                            out=wf[:sz, :], in_=ps_wf[:sz, :], func=AFT.Identity
                        )
                        wf_sb.append(wf)
                    totb = small.tile([GC, G], f32, name="totb")
                    for mc in range(G):
                        msl = slice(mc * GC, (mc + 1) * GC)
                        pb = psB.tile([GC, 1], f32, tag="B", name=f"pb{mc}")
                        for g in range(G):
                            nc.tensor.matmul(
                                pb[:],
                                wt_sb[g][:, msl],
                                bv16_sb[:, g, None],
                                start=(g == 0),
                                stop=(g == G - 1),
                            )
                        nc.vector.tensor_tensor(
                            out=totb[:, mc, None],
                            in0=pb[:],
                            in1=bp_sb[:, mc, None],
                            op=ALU.add,
                        )

                # ---------------- pass 2 ----------------
                if "p2" in stages:
                    for ch in range(NCH):
                        cs = slice(ch * CW, (ch + 1) * CW)
                        ost = [
                            op.tile([GC, CW], f16, tag=f"ost{mc}", name=f"ost{mc}_{ch}")
                            for mc in range(G)
                        ]
                        for bi in range(BPC):
                            bs = slice(bi * BW, (bi + 1) * BW)
                            for mc in range(G):
                                ms = slice(mc * GC, (mc + 1) * GC)
                                pout = psA.tile([GC, BW], f32, tag="A")
                                nc.tensor.matmul(
                                    pout[:],
                                    wf_sb[0][:, ms],
                                    x0_sb[ch][:, bs],
                                    start=True,
                                    stop=False,
                                )
                                nc.tensor.matmul(
                                    pout[:],
                                    wf_sb[1][0:64, ms],
                                    x1_sb[ch][0:64, bs],
                                    start=False,
                                    stop=True,
                                )
                                if mc == 0:
                                    nc.scalar.activation(
                                        out=ost[mc][:, bs],
                                        in_=pout[:],
                                        func=AFT.Identity,
                                        bias=totb[:, mc, None],
                                        scale=1.0,
                                    )
                                else:
                                    nc.vector.tensor_scalar_add(
                                        out=ost[mc][:, bs],
                                        in0=pout[:],
                                        scalar1=totb[:, mc, None],
                                    )
                        if ch == NCH - 1:
                            # stream the tail out per-block so the final drain
                            # overlaps compute instead of waiting on one big DMA
                            for q in range(BPC):
                                qs2 = slice(ch * CW + q * BW, ch * CW + (q + 1) * BW)
                                bs2 = slice(q * BW, (q + 1) * BW)
                                for mc in range(G):
                                    ms = slice(mc * GC, (mc + 1) * GC)
                                    eng = nc.scalar if mc == 0 else nc.sync
                                    eng.dma_start(out[ms, qs2], ost[mc][:, bs2])
                        else:
                            for mc in range(G):
                                ms = slice(mc * GC, (mc + 1) * GC)
                                eng = nc.scalar if mc == 0 else nc.sync
                                eng.dma_start(out[ms, cs], ost[mc][:])
                elif "od" in stages:
                    dummy_o = small.tile([GC, CW], f16, tag="dummy_o")
                    nc.vector.memset(dummy_o[:, 0:1], 0.0)
                    for ch in range(NCH):
                        cs = slice(ch * CW, (ch + 1) * CW)
                        for mc in range(G):
                            ms = slice(mc * GC, (mc + 1) * GC)
                            nc.scalar.dma_start(out[ms, cs], dummy_o[:])

    if split:
        _split_excess_waits(nc)
    return nc


def _host_aux(W_qkv, b_qkv, temperature, W_proj, b_proj):
    import ml_dtypes

    W_qkv = np.asarray(W_qkv, dtype=np.float32)
    b_qkv = np.asarray(b_qkv, dtype=np.float32)
    temperature = np.asarray(temperature, dtype=np.float32).reshape(NHEADS)
    W_proj = np.asarray(W_proj, dtype=np.float32)
    b_proj = np.asarray(b_proj, dtype=np.float32)

    f16 = np.float16
    F8 = ml_dtypes.float8_e4m3

    # Wqk columns permuted group-major: col = g*192 + {q:0,k:1}*96 + i
    wqk = W_qkv[:, 0 : 2 * C]  # [C, 2C], cols = {q:0,k:1}*192 + g*96 + i
    bqk = b_qkv[0 : 2 * C]
    perm = np.concatenate(
        [
            np.arange(tq * 192 + g * 96, tq * 192 + g * 96 + 96)
            for g in range(G)
            for tq in range(2)
        ]
    )
    wqk_p = wqk[:, perm]  # [192, 384]
    bqk_p = bqk[perm]  # [384]
    # rows -> [97, 2]: c = t*96 + p; row 96 = (bias, 0)
    wqk8 = np.zeros((97, 2, 2 * C), np.float32)
    wqk8[0:96, 0, :] = wqk_p[0:96, :]
    wqk8[0:96, 1, :] = wqk_p[96:192, :]
    wqk8[96, 0, :] = bqk_p

    aux = {
        "wqk8": wqk8.astype(F8),
        "wvt": np.ascontiguousarray(
            W_qkv[:, 2 * C : 3 * C].T.reshape(G, GC, C).transpose(1, 0, 2)
        ).astype(f16),
        "wp0": np.ascontiguousarray(W_proj[0:GC, :]).astype(f16),
        "wp1": np.ascontiguousarray(W_proj[GC:C, :]).astype(f16),
        "bv16": np.ascontiguousarray(
            np.stack(
                [b_qkv[2 * C + g * GC : 2 * C + (g + 1) * GC] for g in range(G)],
                axis=1,
            )
        ).astype(f16),
        "bp": np.ascontiguousarray(
            np.stack([b_proj[g * GC : (g + 1) * GC] for g in range(G)], axis=1)
        ),
        "tmp96": np.ascontiguousarray(
            np.stack(
                [np.repeat(temperature[4 * g : 4 * (g + 1)], DH) for g in range(G)],
                axis=1,
            )
        ),
        "eye96": np.eye(GC, dtype=np.float32),
        "bmask": np.where(
            np.kron(np.eye(4, dtype=bool), np.ones((DH, DH), dtype=bool)),
            np.float32(0.0),
            np.float32(NEG_BIG),
        ).astype(np.float32),
    }
    return aux


def make_in_maps(x, W_qkv, b_qkv, temperature, W_proj, b_proj):
    import ml_dtypes

    F8 = ml_dtypes.float8_e4m3
    x = np.asarray(x, dtype=np.float32).reshape(B, C, N)
    x16 = x.astype(np.float16)
    aux = _host_aux(W_qkv, b_qkv, temperature, W_proj, b_proj)
    ones_row16 = np.ones((1, N), dtype=np.float16)

    # fp8 x in [97, 2, N]: c = t*96 + p, row 96 = (1, 0)
    x8_all = np.zeros((B, 97, 2, N), np.float32)
    x8_all[:, 0:96, 0, :] = x[:, 0:96, :]
    x8_all[:, 0:96, 1, :] = x[:, 96:192, :]
    x8_all[:, 96, 0, :] = 1.0
    x8_all = x8_all.astype(F8)

    return [
        {
            "x8": x8_all[b],
            "x0": np.ascontiguousarray(x16[b, 0:128]),
            "x1": np.ascontiguousarray(
                np.concatenate([x16[b, 128:C], ones_row16], axis=0)
            ),
            **aux,
        }
        for b in range(B)
    ]


def kernel(x, W_qkv, b_qkv, temperature, W_proj, b_proj):
    from concourse.bass_utils import run_bass_kernel_spmd

    if "nc" not in _BUILT:
        _BUILT["nc"] = _build(N)
    nc = _BUILT["nc"]

    in_maps = make_in_maps(x, W_qkv, b_qkv, temperature, W_proj, b_proj)
    res = run_bass_kernel_spmd(nc, in_maps, core_ids=list(range(B)))
    out = np.stack([res.results[b]["out"] for b in range(B)], axis=0)
    return out.astype(np.float32).reshape(B, C, HH, WW)
